# revision 23
# baseline (speedup 1.0000x reference)
"""Single-head causal attention on 8 TRN2 NeuronCores.

Problem: x[B=8, T=2048, C=1024], Wq/Wk/Wv[C, H=64] (fp32)
  q = x@Wq; k = x@Wk; v = x@Wv
  wei = softmax(mask(q k^T * C^-0.5)); out = wei @ v       -> [B, T, H]

Sharding: data-parallel over batch, one batch element per core.  The
host hands each core its x slice already C-major (x[b].T) and the
weights packed per 128-c-tile, so the device reads x with the
contraction dim on partitions directly -- no on-device transpose of x.

Per-core dataflow (all matmuls bf16, fp32 PSUM accumulation):
  1. xT [C,T] fp32 --SWDGE cast DMA--> xt bf16 [128, 8, 2048],
     streamed in eight half-chunk pieces so QKV matmuls chase the load.
     Weights ride the otherwise-idle sync HWDGE queue as fp32 and are
     cast once on DVE.
  2. QKV per 512-wide t-chunk: packed [Wq|Wk] stationary -> psum
     [qT;kT] (kT shifted to partitions 0:64 by a DVE copy), Wv -> vT
     -> four PE transposes -> v_nat (no xbar DMA: the xbar locks all
     DMA rings and stalls the x load stream).
  3. S^T blocks = kT.T @ qT (keys on partitions), two s-blocks paired
     per two-bank PSUM tile.  The causal mask is PRELOADED into the
     psum bank by a PE matmul (maskT stationary x shifted identity)
     that opens the accumulation group; the diagonal S matmul then
     accumulates on top (start=False).  This keeps the S->exp chain
     PE->ACT only -- no DVE op in the loop.  ONE exp per pair on
     ScalarE (ACT pays ~400ns fixed cost per instruction); scale
     1/sqrt(C) folds into the exp.  PV: out_un^T[65, T] accumulates
     [v|ones].T @ exp(S^T) -- row 64 = sumexp for free.  Attention
     pairs of neighbouring chunks interleave in the schedule.
  4. PE-transpose out_un^T chunks, multiply by 1/sumexp, DMA out.
"""
import sys

sys.path.insert(0, "/opt/trn_rl_repo")

import numpy as np

import concourse.bass as bass
import concourse.mybir as mybir
import concourse.tile as tile
from concourse import bacc
from concourse.bass_utils import run_bass_kernel_spmd
from concourse.masks import make_identity

B, T, C, H = 8, 2048, 1024, 64
NTT = T // 128   # 16 t-tiles
NCT = C // 128   # 8  c-tiles
NCH = T // 512   # 4  t-chunks (moving free dim)
SCALE = float(C) ** -0.5
MASKVAL = -32768.0  # pre-scale additive mask; * SCALE -> -1024 -> exp -> 0
VP = 80          # v_nat per-tile stride

F32 = mybir.dt.float32
BF16 = mybir.dt.bfloat16


def emit_loads(nc, xD, xtpool):
    # stream x in: SWDGE cast DMAs, half-chunk pieces so the QKV matmuls
    # chase the load front.  Issued before any other gpsimd work except
    # the warmup memset.
    xt = xtpool.tile([128, NCT, T], BF16, tag="xt")
    xR = xD.rearrange("(k p) t -> p k t", p=128)
    for n in range(NCH):
        sl = slice(n * 512, (n + 1) * 512)
        nc.gpsimd.dma_start(xt[:, 0:4, sl], xR[:, 0:4, sl])
        nc.gpsimd.dma_start(xt[:, 4:8, sl], xR[:, 4:8, sl])
    return xt


def emit_body(nc, tc, outD, consts, pools, xt):
    AF = mybir.ActivationFunctionType
    ALU = mybir.AluOpType
    wqkv, maskT, identw, ident, dum = consts
    qkpool, ptpool, opool, fpool = pools

    # ---- QKV projections + attention, pipelined per 512-wide t-chunk ----
    qk_a = qkpool.tile([128, T], BF16, tag="qka")   # rows 0:64 qT, 64:128 kT
    kt_lo = qkpool.tile([64, T], BF16, tag="ktlo")  # kT at partitions 0:64
    vt = qkpool.tile([64, T], F32, tag="vt")        # vT at partitions 0:64
    v_nat = qkpool.tile([128, NTT, VP], BF16, tag="vnat")  # [s_lo, s_hi, v|1]
    nc.gpsimd.memset(v_nat[:, :, H:H + 1], 1.0)
    o_out = fpool.tile([128, NTT, H], F32, tag="oout")
    outR = outD.rearrange("(g p) h -> p g h", p=128)
    with (
        tc.tile_pool(name="qkps", bufs=1, space="PSUM") as qkps,
        tc.tile_pool(name="aux", bufs=1, space="PSUM") as aux,
        tc.tile_pool(name="ops", bufs=2, space="PSUM") as ops,
        tc.tile_pool(name="stps", bufs=2, space="PSUM") as stps,
    ):
        vps = aux   # v-projection psum + v_nat transpose bank
        fps = stps  # fin transposes rotate through the S psum bufs
        # PE warm-up on the zero dummy: ramps the HAM clock-gate to 8/8
        # while the first x chunk is still in flight.
        warm = qkps.tile([128, 512], F32, tag="psqk")
        for _ in range(8):
            nc.tensor.matmul(
                warm[:], dum[:, 0:128], dum[:], start=True, stop=True
            )

        pending_pv = []  # deferred PV matmuls: hide the exp latency

        def flush_pv():
            for mm in pending_pv:
                nc.tensor.matmul(*mm[0], **mm[1])
            pending_pv.clear()

        def emit_qkv(n):
            sl = slice(n * 512, (n + 1) * 512)
            # qk first: the attention-critical chain is qk_a -> kt_lo -> S
            ps_qk = qkps.tile([128, 512], F32, tag="psqk")
            for k in range(NCT):
                nc.tensor.matmul(
                    ps_qk[:], wqkv[:, k, 0:128], xt[:, k, sl],
                    start=(k == 0), stop=(k == NCT - 1),
                )
            nc.vector.tensor_copy(qk_a[:, sl], ps_qk[:])
            # kT shifted to partitions 0:64 on DVE (a DMA here would queue
            # behind the saturated x-load rings for ~8us)
            nc.vector.tensor_copy(kt_lo[:, sl], ps_qk[64:128, :])
            ps_v_t = vps.tile([128, 512], F32, tag="aux")
            ps_v = ps_v_t[0:64, :]
            for k in range(NCT):
                nc.tensor.matmul(
                    ps_v[:], wqkv[:, k, 128:192], xt[:, k, sl],
                    start=(k == 0), stop=(k == NCT - 1),
                )
            nc.vector.tensor_copy(vt[:, sl], ps_v[:])
            # vT -> v_nat via four PE transposes into one accumulation
            # group (disjoint 64-col ranges of one psum bank)
            tr_t = vps.tile([128, 512], F32, tag="aux")
            for rr in range(4):
                nc.tensor.matmul(
                    tr_t[:, rr * H:(rr + 1) * H],
                    vt[:, n * 512 + rr * 128:n * 512 + (rr + 1) * 128],
                    ident[0:64, 0:64],
                    is_transpose=True, start=(rr == 0), stop=(rr == 3),
                )
            nc.vector.tensor_copy(
                v_nat[:, n * 4:(n + 1) * 4, 0:H],
                tr_t[:, 0:4 * H].rearrange("p (r h) -> p r h", h=H),
            )
            flush_pv()

        out_pcs = {}

        def emit_pair(ci, pb):
            # one pair of s-blocks for chunk ci: two S matmuls into a
            # two-bank psum tile, one wide exp, PVs deferred
            nsb = 4 * ci + 4
            if pb == 0:
                out_pcs[ci] = ops.tile([H + 1, 512], F32, tag="outc",
                                       name="out_pc")
            out_pc = out_pcs[ci]
            st = stps.tile([128, 2, 512], F32, tag="st")
            pt = ptpool.tile([128, 2, 512], BF16, tag="pt")
            # plane order: on diagonal pairs put the LATER s-block in
            # plane 0 so the single exp can skip a longer dead prefix
            diag = (2 * pb + 1) - 4 * ci >= 0
            js = (1, 0) if diag else (0, 1)
            t0s = {}
            for j, jj in enumerate(js):
                sb = 2 * pb + jj
                r = sb - 4 * ci  # >=0 on diagonal s-blocks
                t0 = max(r, 0) * 128
                t0s[jj] = t0
                if r >= 0:
                    # causal-mask preload: maskT.T @ shifted-identity
                    # writes MASKVAL above the diagonal (and zeros
                    # elsewhere) and OPENS the psum group; the S matmul
                    # below accumulates on top.
                    nc.tensor.matmul(
                        st[:, j, :],
                        maskT[:],
                        identw[:, 512 - t0:1024 - t0],
                        start=True, stop=False,
                    )
                # diagonal blocks computed full-width: keeps the pair
                # tile fully defined for the single wide exp below
                nc.tensor.matmul(
                    st[:, j, :],
                    kt_lo[:, sb * 128:(sb + 1) * 128],
                    qk_a[0:64, ci * 512:(ci + 1) * 512],
                    start=(r < 0), stop=True,
                )
            # single exp over both banks; leading cols below plane 0's
            # diagonal are dead -> start there
            lead = t0s[js[0]]
            stf = st.rearrange("p a b -> p (a b)")
            ptf = pt.rearrange("p a b -> p (a b)")
            nc.scalar.activation(
                ptf[:, lead:1024], stf[:, lead:1024], AF.Exp, scale=SCALE
            )
            flush_pv()
            for jj in (0, 1):  # PV in sb order: group start flag is on sb 0
                j = js.index(jj)
                sb = 2 * pb + jj
                t0 = t0s[jj]
                pending_pv.append((
                    (out_pc[:, t0:512], v_nat[:, sb, 0:H + 1],
                     pt[:, j, t0:512]),
                    dict(start=(sb == 0), stop=(sb == nsb - 1)),
                ))

        def emit_attn_out(ci):
            # normalize + transpose + store this chunk
            flush_pv()
            out_pc = out_pcs[ci]
            o_c = opool.tile([H + 1, 512], F32, tag="osb")
            nc.vector.tensor_copy(o_c[:], out_pc[:])
            last = ci == NCH - 1
            for rr in range(4):
                tk = ci * 4 + rr
                fin_t = fps.tile([128, 2, 512], F32, tag="st")
                fin = fin_t[:, 0, 0:H + 1]
                nc.tensor.transpose(
                    fin[:],
                    o_c[:, rr * 128:(rr + 1) * 128],
                    ident[0:H + 1, 0:H + 1],
                )
                rcp = fpool.tile([128, 1], F32, tag="rcp")
                nc.vector.reciprocal(rcp[:], fin[:, H:H + 1])
                nc.vector.tensor_scalar_mul(
                    o_out[:, tk, :], fin[:, 0:H], rcp[:]
                )
                if last:
                    # split the final chunk's store so the last piece on
                    # the tail is small
                    nc.gpsimd.dma_start(
                        outR[:, tk:tk + 1, :], o_out[:, tk:tk + 1, :]
                    )
            if not last:
                nc.gpsimd.dma_start(
                    outR[:, ci * 4:(ci + 1) * 4, :],
                    o_out[:, ci * 4:(ci + 1) * 4, :],
                )

        # Schedule: QKV chunks chase the x load; attention pairs of
        # neighbouring chunks interleave so the PE always has independent
        # matmul work while an exp is in flight.  The build-time scheduler
        # models the x load as near-instant and would hoist load-gated QKV
        # matmuls ahead of ready attention work (stalling the in-order PE
        # on real load latency) -- the no_sync_barrier fences pin chunk
        # 2/3 projections behind the attention work that is data-ready
        # first.
        emit_qkv(0)
        emit_qkv(1)
        emit_pair(0, 0)
        emit_pair(0, 1)
        emit_pair(1, 0)
        emit_pair(1, 1)
        emit_pair(1, 2)
        emit_pair(1, 3)
        flush_pv()
        tc.no_sync_barrier()
        emit_qkv(2)
        emit_attn_out(0)
        emit_pair(2, 0)
        emit_pair(2, 1)
        flush_pv()
        tc.no_sync_barrier()
        emit_qkv(3)
        emit_attn_out(1)
        emit_pair(2, 2)
        emit_pair(3, 0)
        emit_pair(2, 3)
        emit_pair(3, 1)
        emit_pair(2, 4)
        emit_pair(3, 2)
        emit_pair(2, 5)
        emit_pair(3, 3)
        emit_attn_out(2)
        emit_pair(3, 4)
        emit_pair(3, 5)
        emit_pair(3, 6)
        emit_pair(3, 7)
        emit_attn_out(3)


def build_nc(reps=1):
    nc = bacc.Bacc("TRN2", target_bir_lowering=False, debug=False)
    xD = nc.dram_tensor("xT", [C, T], F32, kind="ExternalInput").ap()
    wD = nc.dram_tensor("wqkv", [128, NCT, 192], F32,
                        kind="ExternalInput").ap()
    outD = nc.dram_tensor("out", [T, H], F32, kind="ExternalOutput").ap()

    ALU = mybir.AluOpType
    AF = mybir.ActivationFunctionType

    with tile.TileContext(nc) as tc:
        with (
            tc.tile_pool(name="const", bufs=1) as cpool,
            tc.tile_pool(name="xt", bufs=1) as xtpool,
            tc.tile_pool(name="qk", bufs=2) as qkpool,
            tc.tile_pool(name="pt", bufs=4) as ptpool,
            tc.tile_pool(name="osb", bufs=3) as opool,
            tc.tile_pool(name="fin", bufs=2) as fpool,
        ):
            # zero dummy: PE warm-up source + ACT exp-table priming, ready
            # before any DMA lands.
            dum = cpool.tile([128, 512], BF16)
            nc.gpsimd.memset(dum[:], 0.0)
            prim = cpool.tile([128, 1], F32)

            # weights ride the idle sync HWDGE queue as fp32 (bypasses
            # the x-load SWDGE FIFO), cast once on DVE
            wstage = cpool.tile([128, NCT, 192], F32)
            nc.sync.dma_start(wstage[:], wD[:])
            wqkv = cpool.tile([128, NCT, 192], BF16)
            nc.vector.tensor_copy(wqkv[:], wstage[:])

            # x load descriptors next: everything below is engine-side or
            # rides behind them
            xt0 = emit_loads(nc, xD, xtpool)

            # maskT[c, s] = MASKVAL where c < s else 0; mask preload is
            # maskT.T @ shifted-identity
            maskT = cpool.tile([128, 128], BF16)
            nc.gpsimd.memset(maskT[:], 0.0)
            nc.gpsimd.affine_select(
                out=maskT[:], in_=maskT[:],
                compare_op=ALU.is_ge, fill=MASKVAL,
                base=0, pattern=[[-1, 128]], channel_multiplier=1,
            )
            # identw: zeros except an identity block at cols 512:640;
            # slicing [512-t0 : 1024-t0] shifts the written window to t0
            identw = cpool.tile([128, 1024], BF16)
            nc.gpsimd.memset(identw[:], 0.0)
            make_identity(nc, identw[:, 512:640])
            ident = cpool.tile([128, 128], F32)
            make_identity(nc, ident[:])

            # prime the ScalarE activation table before the first real exp
            nc.scalar.activation(prim[:], dum[:, 0:1], AF.Exp, scale=SCALE)

            consts = (wqkv, maskT, identw, ident, dum)
            pools = (qkpool, ptpool, opool, fpool)
            for rep in range(reps):
                xt_cur = xt0 if rep == 0 else emit_loads(nc, xD, xtpool)
                emit_body(nc, tc, outD, consts, pools, xt_cur)

    nc.compile()
    return nc


def _pack_wqkv(Wq, Wk, Wv):
    # [128, NCT, 192]: per c-tile k, cols 0:64 Wq, 64:128 Wk, 128:192 Wv
    w = np.empty((128, NCT, 192), dtype=np.float32)
    for k in range(NCT):
        rows = slice(k * 128, (k + 1) * 128)
        w[:, k, 0:64] = Wq[rows]
        w[:, k, 64:128] = Wk[rows]
        w[:, k, 128:192] = Wv[rows]
    return w


def make_in_maps(x, Wq, Wk, Wv):
    wqkv = _pack_wqkv(
        np.asarray(Wq, dtype=np.float32),
        np.asarray(Wk, dtype=np.float32),
        np.asarray(Wv, dtype=np.float32),
    )
    return [
        {
            "xT": np.ascontiguousarray(
                np.asarray(x[b], dtype=np.float32).T
            ),
            "wqkv": wqkv,
        }
        for b in range(B)
    ]


_NC = None


def kernel(x, Wq, Wk, Wv):
    global _NC
    if _NC is None:
        _NC = build_nc()
    in_maps = make_in_maps(x, Wq, Wk, Wv)
    res = run_bass_kernel_spmd(_NC, in_maps, core_ids=list(range(B)))
    return np.stack([res.results[b]["out"] for b in range(B)], axis=0)


# revision 26
# speedup vs baseline: 1.1808x; 1.1808x over previous
"""Single-head causal attention on 8 TRN2 NeuronCores.

Problem: x[B=8, T=2048, C=1024], Wq/Wk/Wv[C, H=64] (fp32)
  q = x@Wq; k = x@Wk; v = x@Wv
  wei = softmax(mask(q k^T * C^-0.5)); out = wei @ v       -> [B, T, H]

Sharding: data-parallel over batch, one batch element per core.  The
host hands each core its x slice already C-major (x[b].T) and the
weights packed per 128-c-tile, so the device reads x with the
contraction dim on partitions directly -- no on-device transpose of x.

Per-core dataflow (all matmuls bf16, fp32 PSUM accumulation):
  1. xT [C,T] fp32 --SWDGE cast DMA--> xt bf16 [128, 8, 2048],
     streamed in eight half-chunk pieces so QKV matmuls chase the load.
     Weights ride the otherwise-idle sync HWDGE queue as fp32 and are
     cast once on DVE.
  2. QKV per 512-wide t-chunk: packed [Wq|Wk] stationary -> psum
     [qT;kT] (kT shifted to partitions 0:64 by a DVE copy), Wv -> vT
     -> four PE transposes -> v_nat (no xbar DMA: the xbar locks all
     DMA rings and stalls the x load stream).
  3. S^T blocks = kT.T @ qT (keys on partitions), two s-blocks paired
     per two-bank PSUM tile.  The causal mask is PRELOADED into the
     psum bank by a PE matmul (maskT stationary x shifted identity)
     that opens the accumulation group; the diagonal S matmul then
     accumulates on top (start=False) -- the S->exp chain is PE->ACT
     only.  ONE exp per pair on ScalarE (ACT pays ~680ns fixed cost
     per instruction); scale 1/sqrt(C) folds into the exp.  PV:
     out_un^T[65, T] accumulates [v|ones].T @ exp(S^T) -- row 64 =
     sumexp for free.
  4. PE-transpose out_un^T chunks, multiply by 1/sumexp, DMA out.

The build-time Tile scheduler models DMA as near-instant and would
hoist load-gated QKV matmuls ahead of ready attention work, stalling
the in-order PE at runtime.  All PE matmuls are therefore chained with
scheduler-only (no_sync) dependencies in a hand-interleaved order that
keeps ScalarE -- the serial bottleneck of the attention phase -- fed
from the earliest possible S pair to the last.
"""
import sys

sys.path.insert(0, "/opt/trn_rl_repo")

import numpy as np

import concourse.bass as bass
import concourse.mybir as mybir
import concourse.tile as tile
from concourse import bacc
from concourse.bass_utils import run_bass_kernel_spmd
from concourse.instruction_name_ordered_set import InstructionNameOrderedSet
from concourse.masks import make_identity

B, T, C, H = 8, 2048, 1024, 64
NTT = T // 128   # 16 t-tiles
NCT = C // 128   # 8  c-tiles
NCH = T // 512   # 4  t-chunks (moving free dim)
SCALE = float(C) ** -0.5
MASKVAL = -32768.0  # pre-scale additive mask; * SCALE -> -1024 -> exp -> 0
VP = 80          # v_nat per-tile stride

F32 = mybir.dt.float32
BF16 = mybir.dt.bfloat16


def emit_loads(nc, xD, xtpool):
    # stream x in: SWDGE cast DMAs, half-chunk pieces so the QKV matmuls
    # chase the load front.
    xt = xtpool.tile([128, NCT, T], BF16, tag="xt")
    xR = xD.rearrange("(k p) t -> p k t", p=128)
    for n in range(NCH):
        sl = slice(n * 512, (n + 1) * 512)
        nc.gpsimd.dma_start(xt[:, 0:4, sl], xR[:, 0:4, sl])
        nc.gpsimd.dma_start(xt[:, 4:8, sl], xR[:, 4:8, sl])
    return xt


def emit_body(nc, tc, outD, consts, pools, xt):
    AF = mybir.ActivationFunctionType
    ALU = mybir.AluOpType
    wqkv, maskT, identw, ident, dum = consts
    qkpool, ptpool, opool, fpool = pools

    # ---- PE chain: force the PE stream order to the emit order ----
    prev_pe = [None]

    def pe_mm(*args, **kw):
        inst = nc.tensor.matmul(*args, **kw)
        if prev_pe[0] is not None:
            deps = InstructionNameOrderedSet()
            deps.add(prev_pe[0])
            inst.ins.add_nosync_dependencies_from(deps)
        prev_pe[0] = inst.ins.name
        return inst

    # ---- QKV projections + attention, pipelined per 512-wide t-chunk ----
    qk_a = qkpool.tile([128, T], BF16, tag="qka")   # rows 0:64 qT, 64:128 kT
    kt_lo = qkpool.tile([64, T], BF16, tag="ktlo")  # kT at partitions 0:64
    vt = qkpool.tile([64, T], F32, tag="vt")        # vT at partitions 0:64
    v_nat = qkpool.tile([128, NTT, VP], BF16, tag="vnat")  # [s_lo, s_hi, v|1]
    nc.gpsimd.memset(v_nat[:, :, H:H + 1], 1.0)
    o_out = fpool.tile([128, NTT, H], F32, tag="oout")
    outR = outD.rearrange("(g p) h -> p g h", p=128)
    with (
        tc.tile_pool(name="qkps", bufs=1, space="PSUM") as qkps,
        tc.tile_pool(name="aux", bufs=1, space="PSUM") as aux,
        tc.tile_pool(name="ops", bufs=2, space="PSUM") as ops,
        tc.tile_pool(name="stps", bufs=2, space="PSUM") as stps,
    ):
        vps = aux   # v-projection psum + v_nat transpose bank
        # PE warm-up on the zero dummy: ramps the HAM clock-gate to 8/8
        # while the first x chunk is still in flight.
        warm = qkps.tile([128, 512], F32, tag="psqk")
        for _ in range(8):
            pe_mm(warm[:], dum[:, 0:128], dum[:], start=True, stop=True)

        pending_pv = []  # deferred PV matmuls: hide the exp latency

        def flush_pv():
            for mm in pending_pv:
                pe_mm(*mm[0], **mm[1])
            pending_pv.clear()

        def emit_qk(n):
            sl = slice(n * 512, (n + 1) * 512)
            ps_qk = qkps.tile([128, 512], F32, tag="psqk")
            for k in range(NCT):
                pe_mm(
                    ps_qk[:], wqkv[:, k, 0:128], xt[:, k, sl],
                    start=(k == 0), stop=(k == NCT - 1),
                )
            nc.vector.tensor_copy(qk_a[:, sl], ps_qk[:])
            # kT shifted to partitions 0:64 on DVE (a DMA here would queue
            # behind the saturated x-load rings for ~8us)
            nc.vector.tensor_copy(kt_lo[:, sl], ps_qk[64:128, :])

        def emit_v(n):
            sl = slice(n * 512, (n + 1) * 512)
            ps_v_t = vps.tile([128, 512], F32, tag="aux")
            ps_v = ps_v_t[0:64, :]
            for k in range(NCT):
                pe_mm(
                    ps_v[:], wqkv[:, k, 128:192], xt[:, k, sl],
                    start=(k == 0), stop=(k == NCT - 1),
                )
            nc.vector.tensor_copy(vt[:, sl], ps_v[:])

        def emit_vtr(n):
            # vT -> v_nat via four PE transposes into one accumulation
            # group (disjoint 64-col ranges of one psum bank)
            tr_t = vps.tile([128, 512], F32, tag="aux")
            for rr in range(4):
                pe_mm(
                    tr_t[:, rr * H:(rr + 1) * H],
                    vt[:, n * 512 + rr * 128:n * 512 + (rr + 1) * 128],
                    ident[0:64, 0:64],
                    is_transpose=True, start=(rr == 0), stop=(rr == 3),
                )
            nc.vector.tensor_copy(
                v_nat[:, n * 4:(n + 1) * 4, 0:H],
                tr_t[:, 0:4 * H].rearrange("p (r h) -> p r h", h=H),
            )

        out_pcs = {}

        def emit_pair(ci, pb):
            # one pair of s-blocks for chunk ci: two S matmuls into a
            # two-bank psum tile, one wide exp, PVs deferred
            nsb = 4 * ci + 4
            if pb == 0:
                out_pcs[ci] = ops.tile([H + 1, 512], F32, tag="outc",
                                       name="out_pc")
            out_pc = out_pcs[ci]
            st = stps.tile([128, 2, 512], F32, tag="st")
            pt = ptpool.tile([128, 2, 512], BF16, tag="pt")
            # plane order: on diagonal pairs put the LATER s-block in
            # plane 0 so the single exp can skip a longer dead prefix
            diag = (2 * pb + 1) - 4 * ci >= 0
            js = (1, 0) if diag else (0, 1)
            t0s = {}
            for j, jj in enumerate(js):
                sb = 2 * pb + jj
                r = sb - 4 * ci  # >=0 on diagonal s-blocks
                t0 = max(r, 0) * 128
                t0s[jj] = t0
                if r >= 0:
                    # causal-mask preload: maskT.T @ shifted-identity
                    # writes MASKVAL above the diagonal (zeros elsewhere)
                    # and OPENS the psum group; the S matmul accumulates.
                    pe_mm(
                        st[:, j, :],
                        maskT[:],
                        identw[:, 512 - t0:1024 - t0],
                        start=True, stop=False,
                    )
                pe_mm(
                    st[:, j, :],
                    kt_lo[:, sb * 128:(sb + 1) * 128],
                    qk_a[0:64, ci * 512:(ci + 1) * 512],
                    start=(r < 0), stop=True,
                )
            # single exp over both banks; leading cols below plane 0's
            # diagonal are dead -> start there
            lead = t0s[js[0]]
            stf = st.rearrange("p a b -> p (a b)")
            ptf = pt.rearrange("p a b -> p (a b)")
            nc.scalar.activation(
                ptf[:, lead:1024], stf[:, lead:1024], AF.Exp, scale=SCALE
            )
            flush_pv()
            for jj in (0, 1):  # PV in sb order: group start flag is on sb 0
                j = js.index(jj)
                sb = 2 * pb + jj
                t0 = t0s[jj]
                pending_pv.append((
                    (out_pc[:, t0:512], v_nat[:, sb, 0:H + 1],
                     pt[:, j, t0:512]),
                    dict(start=(sb == 0), stop=(sb == nsb - 1)),
                ))

        def emit_attn_out(ci):
            # normalize + transpose + store this chunk; fin transposes
            # alternate between the (idle by now) aux and qkps banks.
            # Chunk ci's PVs were all flushed by later pairs' emissions
            # except for the final chunk.
            if ci == NCH - 1:
                flush_pv()
            out_pc = out_pcs[ci]
            o_c = opool.tile([H + 1, 512], F32, tag="osb")
            nc.vector.tensor_copy(o_c[:], out_pc[:])
            last = ci == NCH - 1
            for rr in range(4):
                tk = ci * 4 + rr
                fpool_ps = vps if rr % 2 == 0 else qkps
                ftag = "aux" if rr % 2 == 0 else "psqk"
                fin_t = fpool_ps.tile([128, 512], F32, tag=ftag,
                                      name="fin_t")
                fin = fin_t[:, 0:H + 1]
                inst = pe_mm(
                    fin[:],
                    o_c[:, rr * 128:(rr + 1) * 128],
                    ident[0:H + 1, 0:H + 1],
                    is_transpose=True,
                )
                rcp = fpool.tile([128, 1], F32, tag="rcp")
                nc.vector.reciprocal(rcp[:], fin[:, H:H + 1])
                nc.vector.tensor_scalar_mul(
                    o_out[:, tk, :], fin[:, 0:H], rcp[:]
                )
                if last:
                    # split the final chunk's store so the last piece on
                    # the tail is small
                    nc.gpsimd.dma_start(
                        outR[:, tk:tk + 1, :], o_out[:, tk:tk + 1, :]
                    )
            if not last:
                nc.gpsimd.dma_start(
                    outR[:, ci * 4:(ci + 1) * 4, :],
                    o_out[:, ci * 4:(ci + 1) * 4, :],
                )

        # Hand-interleaved schedule (see module docstring).  QKV chunks
        # sit exactly where their x pieces land; S pairs start as soon as
        # their kt/q copies allow; ScalarE runs gapless from ~22us.
        # attn_out(ci) must precede the pair that reuses its out_pc bank
        # (ops pool, bufs=2): out0 < P(2,0), out1 < P(3,0).
        emit_qk(0)
        emit_v(0)
        emit_pair(0, 0)
        emit_vtr(0)
        emit_pair(0, 1)
        emit_qk(1)
        emit_v(1)
        emit_pair(1, 0)
        emit_pair(1, 1)
        emit_vtr(1)
        emit_pair(1, 2)
        emit_pair(1, 3)
        emit_qk(2)
        emit_attn_out(0)
        emit_v(2)
        emit_pair(2, 0)
        emit_pair(2, 1)
        emit_vtr(2)
        emit_qk(3)
        emit_pair(2, 2)
        emit_attn_out(1)
        emit_pair(3, 0)
        emit_v(3)
        emit_vtr(3)
        emit_pair(2, 3)
        emit_pair(3, 1)
        emit_pair(2, 4)
        emit_pair(3, 2)
        emit_pair(2, 5)
        emit_pair(3, 3)
        emit_pair(3, 4)
        emit_attn_out(2)
        emit_pair(3, 5)
        emit_pair(3, 6)
        emit_pair(3, 7)
        emit_attn_out(3)


def build_nc(reps=1):
    nc = bacc.Bacc("TRN2", target_bir_lowering=False, debug=False)
    xD = nc.dram_tensor("xT", [C, T], F32, kind="ExternalInput").ap()
    wD = nc.dram_tensor("wqkv", [128, NCT, 192], F32,
                        kind="ExternalInput").ap()
    outD = nc.dram_tensor("out", [T, H], F32, kind="ExternalOutput").ap()

    ALU = mybir.AluOpType
    AF = mybir.ActivationFunctionType

    with tile.TileContext(nc) as tc:
        with (
            tc.tile_pool(name="const", bufs=1) as cpool,
            tc.tile_pool(name="xt", bufs=1) as xtpool,
            tc.tile_pool(name="qk", bufs=2) as qkpool,
            tc.tile_pool(name="pt", bufs=4) as ptpool,
            tc.tile_pool(name="osb", bufs=3) as opool,
            tc.tile_pool(name="fin", bufs=2) as fpool,
        ):
            # zero dummy: PE warm-up source + ACT exp-table priming, ready
            # before any DMA lands.
            dum = cpool.tile([128, 512], BF16)
            nc.gpsimd.memset(dum[:], 0.0)
            prim = cpool.tile([128, 1], F32)

            # weights ride the idle sync HWDGE queue as fp32 (bypasses
            # the x-load SWDGE FIFO), cast once on DVE
            wstage = cpool.tile([128, NCT, 192], F32)
            nc.sync.dma_start(wstage[:], wD[:])
            wqkv = cpool.tile([128, NCT, 192], BF16)
            nc.vector.tensor_copy(wqkv[:], wstage[:])

            # x load descriptors next: everything below is engine-side or
            # rides behind them
            xt0 = emit_loads(nc, xD, xtpool)

            # maskT[c, s] = MASKVAL where c < s else 0; mask preload is
            # maskT.T @ shifted-identity
            maskT = cpool.tile([128, 128], BF16)
            nc.gpsimd.memset(maskT[:], 0.0)
            nc.gpsimd.affine_select(
                out=maskT[:], in_=maskT[:],
                compare_op=ALU.is_ge, fill=MASKVAL,
                base=0, pattern=[[-1, 128]], channel_multiplier=1,
            )
            # identw: zeros except an identity block at cols 512:640;
            # slicing [512-t0 : 1024-t0] shifts the written window to t0
            identw = cpool.tile([128, 1024], BF16)
            nc.gpsimd.memset(identw[:], 0.0)
            make_identity(nc, identw[:, 512:640])
            ident = cpool.tile([128, 128], F32)
            make_identity(nc, ident[:])

            # prime the ScalarE activation table before the first real exp
            nc.scalar.activation(prim[:], dum[:, 0:1], AF.Exp, scale=SCALE)

            consts = (wqkv, maskT, identw, ident, dum)
            pools = (qkpool, ptpool, opool, fpool)
            for rep in range(reps):
                xt_cur = xt0 if rep == 0 else emit_loads(nc, xD, xtpool)
                emit_body(nc, tc, outD, consts, pools, xt_cur)

    nc.compile()
    return nc


def _pack_wqkv(Wq, Wk, Wv):
    # [128, NCT, 192]: per c-tile k, cols 0:64 Wq, 64:128 Wk, 128:192 Wv
    w = np.empty((128, NCT, 192), dtype=np.float32)
    for k in range(NCT):
        rows = slice(k * 128, (k + 1) * 128)
        w[:, k, 0:64] = Wq[rows]
        w[:, k, 64:128] = Wk[rows]
        w[:, k, 128:192] = Wv[rows]
    return w


def make_in_maps(x, Wq, Wk, Wv):
    wqkv = _pack_wqkv(
        np.asarray(Wq, dtype=np.float32),
        np.asarray(Wk, dtype=np.float32),
        np.asarray(Wv, dtype=np.float32),
    )
    return [
        {
            "xT": np.ascontiguousarray(
                np.asarray(x[b], dtype=np.float32).T
            ),
            "wqkv": wqkv,
        }
        for b in range(B)
    ]


_NC = None


def kernel(x, Wq, Wk, Wv):
    global _NC
    if _NC is None:
        _NC = build_nc()
    in_maps = make_in_maps(x, Wq, Wk, Wv)
    res = run_bass_kernel_spmd(_NC, in_maps, core_ids=list(range(B)))
    return np.stack([res.results[b]["out"] for b in range(B)], axis=0)


# revision 32
# speedup vs baseline: 1.2051x; 1.0206x over previous
"""Single-head causal attention on 8 TRN2 NeuronCores.

Problem: x[B=8, T=2048, C=1024], Wq/Wk/Wv[C, H=64] (fp32)
  q = x@Wq; k = x@Wk; v = x@Wv
  wei = softmax(mask(q k^T * C^-0.5)); out = wei @ v       -> [B, T, H]

Sharding: data-parallel over batch, one batch element per core.  The
host hands each core its x slice already C-major (x[b].T) and the
weights packed per 128-c-tile, so the device reads x with the
contraction dim on partitions directly -- no on-device transpose of x.

Per-core dataflow (all matmuls bf16, fp32 PSUM accumulation):
  1. xT [C,T] fp32 --SWDGE cast DMA--> xt bf16 [128, 8, 2048],
     streamed in eight half-chunk pieces so QKV matmuls chase the load.
     Weights ride the otherwise-idle sync HWDGE queue as fp32 and are
     cast once on DVE.
  2. QKV per 512-wide t-chunk: packed [Wq|Wk] stationary -> psum
     [qT;kT] (kT shifted to partitions 0:64 by a DVE copy), Wv -> vT
     -> four PE transposes -> v_nat (no xbar DMA: the xbar locks all
     DMA rings and stalls the x load stream).
  3. S^T blocks = kT.T @ qT (keys on partitions), two s-blocks paired
     per two-bank PSUM tile.  The causal mask is PRELOADED into the
     psum bank by a PE matmul (maskT stationary x shifted identity)
     that opens the accumulation group; the diagonal S matmul then
     accumulates on top (start=False) -- the S->exp chain is PE->ACT
     only.  ONE exp per pair on ScalarE (ACT pays ~680ns fixed cost
     per instruction); scale 1/sqrt(C) folds into the exp.  PV:
     out_un^T[65, T] accumulates [v|ones].T @ exp(S^T) -- row 64 =
     sumexp for free.
  4. PE-transpose out_un^T chunks, multiply by 1/sumexp, DMA out.

The build-time Tile scheduler models DMA as near-instant and would
hoist load-gated QKV matmuls ahead of ready attention work, stalling
the in-order PE at runtime.  All PE matmuls are therefore chained with
scheduler-only (no_sync) dependencies in a hand-interleaved order that
keeps ScalarE -- the serial bottleneck of the attention phase -- fed
from the earliest possible S pair to the last.
"""
import sys

sys.path.insert(0, "/opt/trn_rl_repo")

import numpy as np

import concourse.bass as bass
import concourse.mybir as mybir
import concourse.tile as tile
from concourse import bacc
from concourse.bass_utils import run_bass_kernel_spmd
from concourse.instruction_name_ordered_set import InstructionNameOrderedSet
from concourse.masks import make_identity

B, T, C, H = 8, 2048, 1024, 64
NTT = T // 128   # 16 t-tiles
NCT = C // 128   # 8  c-tiles
NCH = T // 512   # 4  t-chunks (moving free dim)
SCALE = float(C) ** -0.5
MASKVAL = -32768.0  # pre-scale additive mask; * SCALE -> -1024 -> exp -> 0
VP = 80          # v_nat per-tile stride

F32 = mybir.dt.float32
BF16 = mybir.dt.bfloat16


def emit_loads(nc, xD, xtpool):
    # stream x in: SWDGE cast DMAs, half-chunk pieces so the QKV matmuls
    # chase the load front.
    xt = xtpool.tile([128, NCT, T], BF16, tag="xt")
    xR = xD.rearrange("(k p) t -> p k t", p=128)
    for n in range(NCH):
        sl = slice(n * 512, (n + 1) * 512)
        nc.gpsimd.dma_start(xt[:, 0:4, sl], xR[:, 0:4, sl])
        nc.gpsimd.dma_start(xt[:, 4:8, sl], xR[:, 4:8, sl])
    return xt


def emit_body(nc, tc, outD, consts, pools, xt):
    AF = mybir.ActivationFunctionType
    ALU = mybir.AluOpType
    wqkv, maskT, identw, ident, dum = consts
    qkpool, ptpool, opool, fpool = pools

    # ---- PE chain: force the PE stream order to the emit order ----
    prev_pe = [None]

    def pe_mm(*args, **kw):
        inst = nc.tensor.matmul(*args, **kw)
        if prev_pe[0] is not None:
            deps = InstructionNameOrderedSet()
            deps.add(prev_pe[0])
            inst.ins.add_nosync_dependencies_from(deps)
        prev_pe[0] = inst.ins.name
        return inst

    # ---- QKV projections + attention, pipelined per 512-wide t-chunk ----
    qk_a = qkpool.tile([128, T], BF16, tag="qka")   # rows 0:64 qT, 64:128 kT
    kt_lo = qkpool.tile([64, T], BF16, tag="ktlo")  # kT at partitions 0:64
    vt = qkpool.tile([64, T], F32, tag="vt")        # vT at partitions 0:64
    v_nat = qkpool.tile([128, NTT, VP], BF16, tag="vnat")  # [s_lo, s_hi, v|1]
    nc.gpsimd.memset(v_nat[:, :, H:H + 1], 1.0)
    o_out = fpool.tile([128, NTT, H], F32, tag="oout")
    # out DRAM layout is partition-major [128, NTT, H]: per-partition
    # contiguous store descriptors (the host transposes on unshard)
    outR = outD
    with (
        tc.tile_pool(name="qkps", bufs=1, space="PSUM") as qkps,
        tc.tile_pool(name="aux", bufs=1, space="PSUM") as aux,
        tc.tile_pool(name="ops", bufs=2, space="PSUM") as ops,
        tc.tile_pool(name="stps", bufs=2, space="PSUM") as stps,
    ):
        vps = aux   # v-projection psum + v_nat transpose bank
        # PE warm-up on the zero dummy: ramps the HAM clock-gate to 8/8
        # while the first x chunk is still in flight.
        warm = qkps.tile([128, 512], F32, tag="psqk")
        for _ in range(12):
            pe_mm(warm[:], dum[:, 0:128], dum[:], start=True, stop=True)

        pending_pv = []  # deferred PV matmuls: hide the exp latency

        def flush_pv():
            for mm in pending_pv:
                pe_mm(*mm[0], **mm[1])
            pending_pv.clear()

        def emit_qk(n):
            sl = slice(n * 512, (n + 1) * 512)
            ps_qk = qkps.tile([128, 512], F32, tag="psqk")
            for k in range(NCT):
                pe_mm(
                    ps_qk[:], wqkv[:, k, 0:128], xt[:, k, sl],
                    start=(k == 0), stop=(k == NCT - 1),
                )
            nc.vector.tensor_copy(qk_a[:, sl], ps_qk[:])
            # kT shifted to partitions 0:64 on DVE (a DMA here would queue
            # behind the saturated x-load rings for ~8us)
            nc.vector.tensor_copy(kt_lo[:, sl], ps_qk[64:128, :])

        def emit_v(n):
            sl = slice(n * 512, (n + 1) * 512)
            ps_v_t = vps.tile([128, 512], F32, tag="aux")
            ps_v = ps_v_t[0:64, :]
            for k in range(NCT):
                pe_mm(
                    ps_v[:], wqkv[:, k, 128:192], xt[:, k, sl],
                    start=(k == 0), stop=(k == NCT - 1),
                )
            nc.vector.tensor_copy(vt[:, sl], ps_v[:])

        def emit_vtr(n):
            # vT -> v_nat via four PE transposes into one accumulation
            # group (disjoint 64-col ranges of one psum bank)
            tr_t = vps.tile([128, 512], F32, tag="aux")
            for rr in range(4):
                pe_mm(
                    tr_t[:, rr * H:(rr + 1) * H],
                    vt[:, n * 512 + rr * 128:n * 512 + (rr + 1) * 128],
                    ident[0:64, 0:64],
                    is_transpose=True, start=(rr == 0), stop=(rr == 3),
                )
            nc.vector.tensor_copy(
                v_nat[:, n * 4:(n + 1) * 4, 0:H],
                tr_t[:, 0:4 * H].rearrange("p (r h) -> p r h", h=H),
            )

        out_pcs = {}

        def emit_pair(ci, pb):
            # one pair of s-blocks for chunk ci: two S matmuls into a
            # two-bank psum tile, one wide exp, PVs deferred
            nsb = 4 * ci + 4
            if pb == 0:
                out_pcs[ci] = ops.tile([H + 1, 512], F32, tag="outc",
                                       name="out_pc")
            out_pc = out_pcs[ci]
            st = stps.tile([128, 2, 512], F32, tag="st")
            pt = ptpool.tile([128, 2, 512], BF16, tag="pt")
            # plane order: on diagonal pairs put the LATER s-block in
            # plane 0 so the single exp can skip a longer dead prefix
            diag = (2 * pb + 1) - 4 * ci >= 0
            js = (1, 0) if diag else (0, 1)
            t0s = {}
            for j, jj in enumerate(js):
                sb = 2 * pb + jj
                r = sb - 4 * ci  # >=0 on diagonal s-blocks
                t0 = max(r, 0) * 128
                t0s[jj] = t0
                if r >= 0:
                    # causal-mask preload: maskT.T @ shifted-identity
                    # writes MASKVAL above the diagonal (zeros elsewhere)
                    # and OPENS the psum group; the S matmul accumulates.
                    pe_mm(
                        st[:, j, :],
                        maskT[:],
                        identw[:, 512 - t0:1024 - t0],
                        start=True, stop=False,
                    )
                pe_mm(
                    st[:, j, :],
                    kt_lo[:, sb * 128:(sb + 1) * 128],
                    qk_a[0:64, ci * 512:(ci + 1) * 512],
                    start=(r < 0), stop=True,
                )
            # single exp over both banks; leading cols below plane 0's
            # diagonal are dead -> start there
            lead = t0s[js[0]]
            stf = st.rearrange("p a b -> p (a b)")
            ptf = pt.rearrange("p a b -> p (a b)")
            nc.scalar.activation(
                ptf[:, lead:1024], stf[:, lead:1024], AF.Exp, scale=SCALE
            )
            flush_pv()
            for jj in (0, 1):  # PV in sb order: group start flag is on sb 0
                j = js.index(jj)
                sb = 2 * pb + jj
                t0 = t0s[jj]
                pending_pv.append((
                    (out_pc[:, t0:512], v_nat[:, sb, 0:H + 1],
                     pt[:, j, t0:512]),
                    dict(start=(sb == 0), stop=(sb == nsb - 1)),
                ))

        def emit_attn_out(ci):
            # normalize + transpose + store this chunk; fin transposes
            # alternate between the (idle by now) aux and qkps banks.
            # Chunk ci's PVs were all flushed by later pairs' emissions
            # except for the final chunk.
            if ci == NCH - 1:
                flush_pv()
            out_pc = out_pcs[ci]
            o_c = opool.tile([H + 1, 512], F32, tag="osb")
            nc.vector.tensor_copy(o_c[:], out_pc[:])
            last = ci == NCH - 1
            for rr in range(4):
                tk = ci * 4 + rr
                fpool_ps = vps if rr % 2 == 0 else qkps
                ftag = "aux" if rr % 2 == 0 else "psqk"
                fin_t = fpool_ps.tile([128, 512], F32, tag=ftag,
                                      name="fin_t")
                fin = fin_t[:, 0:H + 1]
                inst = pe_mm(
                    fin[:],
                    o_c[:, rr * 128:(rr + 1) * 128],
                    ident[0:H + 1, 0:H + 1],
                    is_transpose=True,
                )
                rcp = fpool.tile([128, 1], F32, tag="rcp")
                nc.vector.reciprocal(rcp[:], fin[:, H:H + 1])
                nc.vector.tensor_scalar_mul(
                    o_out[:, tk, :], fin[:, 0:H], rcp[:]
                )
                if last:
                    # split the final chunk's store so the last piece on
                    # the tail is small (desc-gen is ~0.3us per store)
                    nc.gpsimd.dma_start(
                        outR[:, tk:tk + 1, :], o_out[:, tk:tk + 1, :]
                    )
            if not last:
                nc.gpsimd.dma_start(
                    outR[:, ci * 4:(ci + 1) * 4, :],
                    o_out[:, ci * 4:(ci + 1) * 4, :],
                )

        # Hand-interleaved schedule (see module docstring).  QKV chunks
        # sit exactly where their x pieces land; S pairs start as soon as
        # their kt/q copies allow; ScalarE runs gapless from ~22us.
        # attn_out(ci) must precede the pair that reuses its out_pc bank
        # (ops pool, bufs=2): out0 < P(2,0), out1 < P(3,0).
        emit_qk(0)
        emit_v(0)
        emit_pair(0, 0)
        emit_vtr(0)
        emit_pair(0, 1)
        emit_qk(1)
        emit_v(1)
        emit_pair(1, 0)
        emit_pair(1, 1)
        emit_vtr(1)
        emit_qk(2)
        emit_pair(1, 2)
        emit_pair(1, 3)
        emit_attn_out(0)
        emit_v(2)
        emit_pair(2, 0)
        emit_pair(2, 1)
        emit_vtr(2)
        emit_qk(3)
        emit_pair(2, 2)
        emit_attn_out(1)
        emit_pair(3, 0)
        emit_v(3)
        emit_vtr(3)
        emit_pair(2, 3)
        emit_pair(3, 1)
        emit_pair(2, 4)
        emit_pair(3, 2)
        emit_pair(2, 5)
        emit_pair(3, 3)
        emit_pair(3, 4)
        emit_attn_out(2)
        emit_pair(3, 5)
        emit_pair(3, 6)
        emit_pair(3, 7)
        emit_attn_out(3)


def build_nc(reps=1):
    nc = bacc.Bacc("TRN2", target_bir_lowering=False, debug=False)
    xD = nc.dram_tensor("xT", [C, T], F32, kind="ExternalInput").ap()
    wD = nc.dram_tensor("wqkv", [128, NCT, 192], F32,
                        kind="ExternalInput").ap()
    outD = nc.dram_tensor("out", [128, NTT, H], F32,
                          kind="ExternalOutput").ap()

    ALU = mybir.AluOpType
    AF = mybir.ActivationFunctionType

    with tile.TileContext(nc) as tc:
        with (
            tc.tile_pool(name="const", bufs=1) as cpool,
            tc.tile_pool(name="xt", bufs=1) as xtpool,
            tc.tile_pool(name="qk", bufs=2) as qkpool,
            tc.tile_pool(name="pt", bufs=4) as ptpool,
            tc.tile_pool(name="osb", bufs=3) as opool,
            tc.tile_pool(name="fin", bufs=2) as fpool,
        ):
            # zero dummy: PE warm-up source + ACT exp-table priming, ready
            # before any DMA lands.
            dum = cpool.tile([128, 512], BF16)
            nc.gpsimd.memset(dum[:], 0.0)
            prim = cpool.tile([128, 1], F32)

            # weights ride the idle sync HWDGE queue as fp32 (bypasses
            # the x-load SWDGE FIFO), cast once on DVE
            wstage = cpool.tile([128, NCT, 192], F32)
            nc.sync.dma_start(wstage[:], wD[:])
            wqkv = cpool.tile([128, NCT, 192], BF16)
            nc.vector.tensor_copy(wqkv[:], wstage[:])

            # x load descriptors next: everything below is engine-side or
            # rides behind them
            xt0 = emit_loads(nc, xD, xtpool)

            # maskT[c, s] = MASKVAL where c < s else 0; mask preload is
            # maskT.T @ shifted-identity
            maskT = cpool.tile([128, 128], BF16)
            nc.gpsimd.memset(maskT[:], 0.0)
            nc.gpsimd.affine_select(
                out=maskT[:], in_=maskT[:],
                compare_op=ALU.is_ge, fill=MASKVAL,
                base=0, pattern=[[-1, 128]], channel_multiplier=1,
            )
            # identw: zeros except an identity block at cols 512:640;
            # slicing [512-t0 : 1024-t0] shifts the written window to t0
            identw = cpool.tile([128, 1024], BF16)
            nc.gpsimd.memset(identw[:], 0.0)
            make_identity(nc, identw[:, 512:640])
            ident = cpool.tile([128, 128], F32)
            make_identity(nc, ident[:])

            # prime the ScalarE activation table before the first real exp
            nc.scalar.activation(prim[:], dum[:, 0:1], AF.Exp, scale=SCALE)

            consts = (wqkv, maskT, identw, ident, dum)
            pools = (qkpool, ptpool, opool, fpool)
            for rep in range(reps):
                xt_cur = xt0 if rep == 0 else emit_loads(nc, xD, xtpool)
                emit_body(nc, tc, outD, consts, pools, xt_cur)

    nc.compile()
    return nc


def _pack_wqkv(Wq, Wk, Wv):
    # [128, NCT, 192]: per c-tile k, cols 0:64 Wq, 64:128 Wk, 128:192 Wv
    w = np.empty((128, NCT, 192), dtype=np.float32)
    for k in range(NCT):
        rows = slice(k * 128, (k + 1) * 128)
        w[:, k, 0:64] = Wq[rows]
        w[:, k, 64:128] = Wk[rows]
        w[:, k, 128:192] = Wv[rows]
    return w


def make_in_maps(x, Wq, Wk, Wv):
    wqkv = _pack_wqkv(
        np.asarray(Wq, dtype=np.float32),
        np.asarray(Wk, dtype=np.float32),
        np.asarray(Wv, dtype=np.float32),
    )
    return [
        {
            "xT": np.ascontiguousarray(
                np.asarray(x[b], dtype=np.float32).T
            ),
            "wqkv": wqkv,
        }
        for b in range(B)
    ]


_NC = None


def kernel(x, Wq, Wk, Wv):
    global _NC
    if _NC is None:
        _NC = build_nc()
    in_maps = make_in_maps(x, Wq, Wk, Wv)
    res = run_bass_kernel_spmd(_NC, in_maps, core_ids=list(range(B)))
    # device stores out as [p, g, h]; t = g*128 + p
    return np.stack(
        [
            np.ascontiguousarray(
                np.transpose(res.results[b]["out"], (1, 0, 2))
            ).reshape(T, H)
            for b in range(B)
        ],
        axis=0,
    )
